# revision 28
# baseline (speedup 1.0000x reference)
"""Trainium2 Bass kernel for nn_MMHA_78039555768536.

Gated mix of per-segment causal softmax attention and a linear-attention
memory (delta rule, memory summed over batch per segment).

Strategy (8 cores): reformulate the memory recurrence as a linear matrix
recurrence  M_{t+1} = A_t M_t + B_t  with
    A_t = I - sum_b sk_b^T diag(1/d_b) sk_b   (symmetric)
    B_t = sum_b sk_b^T v_b
Core c owns segments {2c, 2c+1}.  Two all-gathers:
 AG1: per-segment colsums of sk (for the z prefix)  [tiny, hidden under
      the v-projection sweep]
 AG2: per-core pair composition (Abar^T, Bbar)      [1 MB bf16 per rank,
      hidden under the q-projection + attention sweep]
Every core redundantly runs the 7-step pair chain and selects its own
prefix M via a per-core one-hot (SPMD, no branches).

v2 perf notes vs the first working version:
 - all softmax/memread denominators collected into one [72,512] tile and
   inverted with a single DVE reciprocal (was 80 reciprocals at 3us each)
 - denominator broadcast via batched gpsimd DMAs, overlapped with the
   serial chain
 - k projected once (transposed), k-natural recovered with PE transposes
 - A_t computed upper-triangular only (symmetry), identity folded in so
   the chain needs no ID matmuls; M+B additions fused into DVE reads of
   PSUM
 - Wd projection emitted transposed (free dim 512); host un-transposes
 - attention softly pipelined one head ahead to keep the PE p-state high
"""

import sys

sys.path.insert(0, "/opt/trn_rl_repo")

from contextlib import ExitStack

import numpy as np
import ml_dtypes

import concourse.bass as bass
import concourse.bacc as bacc
import concourse.tile as tile
from concourse import mybir
from concourse import bass_utils

B, L, DIN = 4, 8192, 512
H, D, SEG = 8, 64, 512
HD = H * D
NSEG = L // SEG          # 16
NC = 8                   # cores
SPC = NSEG // NC         # segments per core = 2
P = 128
NB = HD // P             # 4 blocks of 128
BS = B * SPC             # batch-segment units per core = 8

bf = mybir.dt.bfloat16
f32 = mybir.dt.float32
AF = mybir.ActivationFunctionType
OP = mybir.AluOpType
bf_np = ml_dtypes.bfloat16

_CACHE = {}


def _build():
    nc = bacc.Bacc(
        "TRN2",
        target_bir_lowering=False,
        debug=False,
        enable_asserts=False,
        num_devices=NC,
    )

    # ---------------- DRAM I/O ----------------
    xt_d = nc.dram_tensor("xt", [B, SPC, NB, P, SEG], bf, kind="ExternalInput").ap()
    wq_d = nc.dram_tensor("wq", [NB, P, HD], bf, kind="ExternalInput").ap()
    wk_d = nc.dram_tensor("wk", [NB, P, HD], bf, kind="ExternalInput").ap()
    wv_d = nc.dram_tensor("wv", [NB, P, HD], bf, kind="ExternalInput").ap()
    wd_d = nc.dram_tensor("wd", [NB, P, D], bf, kind="ExternalInput").ap()
    gcol_d = nc.dram_tensor("gcol", [P, NB], f32, kind="ExternalInput").ap()
    omg_d = nc.dram_tensor("omg", [P, NB], f32, kind="ExternalInput").ap()
    zmask_d = nc.dram_tensor("zmask", [64, NC], f32, kind="ExternalInput").ap()
    oh_d = nc.dram_tensor("oh", [P, NC], f32, kind="ExternalInput").ap()
    mask_d = nc.dram_tensor("cmask", [P, P], bf, kind="ExternalInput").ap()
    ident_d = nc.dram_tensor("ident", [P, P], bf, kind="ExternalInput").ap()
    # transposed output: [D, SEG] per (b, j); host un-transposes
    out_d = nc.dram_tensor("out", [B, SPC, D, SEG], f32, kind="ExternalOutput").ap()

    NDN = BS * H + BS    # 72 denominator rows (64 attn + 8 memread)

    with tile.TileContext(nc) as tc, ExitStack() as ctx:
        const = ctx.enter_context(tc.tile_pool(name="const", bufs=1))
        dram = ctx.enter_context(tc.tile_pool(name="dram", bufs=1, space="DRAM"))
        sing = ctx.enter_context(tc.tile_pool(name="sing", bufs=1))

        WQ = const.tile([P, NB, HD], bf)
        WK = const.tile([P, NB, HD], bf)
        WV = const.tile([P, NB, HD], bf)
        WD = const.tile([P, NB, D], bf)
        GC = const.tile([P, NB], f32)
        OMG = const.tile([P, NB], f32)
        ZM = const.tile([64, NC], f32)
        OH = const.tile([P, NC], f32)
        CM = const.tile([P, P], bf)
        ID = const.tile([P, P], bf)
        ONE = const.tile([P, 1], bf)

        nc.sync.dma_start(out=WQ, in_=wq_d.rearrange("kb p n -> p kb n"))
        nc.sync.dma_start(out=WK, in_=wk_d.rearrange("kb p n -> p kb n"))
        nc.sync.dma_start(out=WV, in_=wv_d.rearrange("kb p n -> p kb n"))
        nc.sync.dma_start(out=WD, in_=wd_d.rearrange("kb p n -> p kb n"))
        nc.sync.dma_start(out=GC, in_=gcol_d)
        nc.sync.dma_start(out=OMG, in_=omg_d)
        nc.sync.dma_start(out=ZM, in_=zmask_d)
        nc.sync.dma_start(out=OH, in_=oh_d)
        nc.sync.dma_start(out=CM, in_=mask_d)
        nc.sync.dma_start(out=ID, in_=ident_d)
        nc.vector.memset(ONE, 1.0)

        # collective bounce buffers
        cs_in = dram.tile([BS, HD], f32)
        cs_out = dram.tile([NC * BS, HD], f32, addr_space="Shared")
        ab_in = dram.tile([2, HD, HD], bf)
        ab_out = dram.tile([NC, 2, HD, HD], bf, addr_space="Shared")
        zrow_d = dram.tile([BS, HD], bf)
        dn_d = dram.tile([NDN, SEG], bf)

        # cross-phase singles
        ZCOL = sing.tile([P, NB, BS], bf)
        AT0 = sing.tile([P, NB, HD], bf)    # I - K of local segment 0
        BT0 = sing.tile([P, NB, HD], bf)
        MSEL = sing.tile([P, NB, HD], bf)   # selected M at segment 2c
        MLOC1 = sing.tile([P, NB, HD], bf)  # M at segment 2c+1
        DN = sing.tile([NDN, SEG], f32)     # raw denominators
        DNR = sing.tile([NDN, SEG], f32)    # reciprocals

        def bs_of(b, j):
            return j * B + b

        # persistent per-bs tensors (sk slots are reused as sq in sweep 2)
        keep = ctx.enter_context(tc.tile_pool(name="keep", bufs=BS))
        skT = [keep.tile([P, NB, SEG], bf, tag="sk", name=f"sk{i}") for i in range(BS)]
        khT = [keep.tile([P, NB, SEG], bf, tag="kh", name=f"kh{i}") for i in range(BS)]
        vaT = [keep.tile([P, NB, H, D + 1], bf, tag="va", name=f"va{i}")
               for i in range(BS)]
        stT = [keep.tile([P, NB, SEG], bf, tag="st", name=f"st{i}") for i in range(BS)]
        sqT = skT  # reuse storage: sk dead after A/B, sq born in sweep 2

        # ======== sweep K: kT projection, k-nat via PE transpose, sk, cs ====
        with tc.tile_pool(name="pxk", bufs=2) as pxk, \
             tc.tile_pool(name="psk", bufs=2, space="PSUM") as psk, \
             tc.tile_pool(name="pskn", bufs=2, space="PSUM") as pskn, \
             tc.tile_pool(name="pscs", bufs=2, space="PSUM") as pscs:
            for j in range(SPC):
                for b in range(B):
                    i = bs_of(b, j)
                    XT = pxk.tile([P, NB, SEG], bf, tag="xt")
                    nc.sync.dma_start(out=XT, in_=xt_d[b, j].rearrange("kb p s -> p kb s"))
                    kh_i = khT[i]
                    for mb in range(NB):
                        pk = psk.tile([P, SEG], f32, tag="pk")
                        for kb in range(NB):
                            nc.tensor.matmul(
                                pk, lhsT=WK[:, kb, mb * P:(mb + 1) * P],
                                rhs=XT[:, kb, :],
                                start=(kb == 0), stop=(kb == NB - 1),
                            )
                        nc.scalar.activation(kh_i[:, mb, :], pk, AF.Copy)
                    sk_i = skT[i]
                    for sb in range(NB):
                        # k natural block row sb from transposes of kh
                        pkn = pskn.tile([P, SEG], bf, tag="pkn")
                        for mb in range(NB):
                            nc.tensor.transpose(
                                pkn[:, mb * P:(mb + 1) * P],
                                kh_i[:, mb, sb * P:(sb + 1) * P], ID,
                            )
                        # elu1(k) = max(k + 1, exp(min(k, 0)))
                        em = pxk.tile([P, SEG], bf, tag="em")
                        nc.vector.tensor_scalar_min(em, pkn, 0.0)
                        ee = pxk.tile([P, SEG], bf, tag="ee")
                        nc.scalar.activation(ee, em, AF.Exp)
                        nc.vector.scalar_tensor_tensor(
                            out=sk_i[:, sb, :], in0=pkn, scalar=1.0, in1=ee,
                            op0=OP.add, op1=OP.max,
                        )
                    pc = pscs.tile([1, HD], f32, tag="pc")
                    for sb in range(NB):
                        nc.tensor.matmul(
                            pc, lhsT=ONE, rhs=sk_i[:, sb, :],
                            start=(sb == 0), stop=(sb == NB - 1),
                        )
                    cs_sb = pxk.tile([1, HD], f32, tag="cs")
                    nc.scalar.activation(cs_sb, pc, AF.Copy)
                    nc.sync.dma_start(out=cs_in[i:i + 1, :], in_=cs_sb)

        # ======== AG1 (hidden under sweep V) ========
        nc.gpsimd.collective_compute(
            "AllGather", OP.bypass,
            replica_groups=[list(range(NC))],
            ins=[cs_in.opt()], outs=[cs_out.opt()],
        )

        # ======== sweep V: v projection ========
        with tc.tile_pool(name="pxv", bufs=2) as pxv, \
             tc.tile_pool(name="psv", bufs=2, space="PSUM") as psv:
            for j in range(SPC):
                for b in range(B):
                    i = bs_of(b, j)
                    XT = pxv.tile([P, NB, SEG], bf, tag="xt")
                    nc.sync.dma_start(out=XT, in_=xt_d[b, j].rearrange("kb p s -> p kb s"))
                    va = vaT[i]
                    nc.vector.memset(va[:, :, :, D:D + 1], 1.0)
                    for sb in range(NB):
                        pv = psv.tile([P, SEG], f32, tag="pv")
                        for kb in range(NB):
                            nc.tensor.matmul(
                                pv, lhsT=XT[:, kb, sb * P:(sb + 1) * P],
                                rhs=WV[:, kb, :],
                                start=(kb == 0), stop=(kb == NB - 1),
                            )
                        nc.vector.tensor_copy(
                            va[:, sb, :, 0:D], pv.rearrange("p (h d) -> p h d", h=H)
                        )

        # ======== per segment: B_t first (AG1-independent), then z prefix,
        # then d/skd + A_t (triangular) ========
        at1 = bt1 = None
        with tc.tile_pool(name="pab", bufs=1) as pab:
            bt_ts = []
            at_ts = []
            for j in range(SPC):
                at_ts.append(pab.tile([P, NB, HD], bf, tag="at", name=f"at{j}")
                             if j > 0 else AT0)
                bt_ts.append(pab.tile([P, NB, HD], bf, tag="bt", name=f"bt{j}")
                             if j > 0 else BT0)
            at1, bt1 = at_ts[1], bt_ts[1]

            # ---- B_t = sum sk^T v for both segments (fills the AG1 window) --
            with tc.tile_pool(name="psbb", bufs=2, space="PSUM") as psbb:
                for j in range(SPC):
                    bt_t = bt_ts[j]
                    for mb in range(NB):
                        pB = psbb.tile([P, HD], f32, tag="pp")
                        n = 0
                        for b in range(B):
                            for sb in range(NB):
                                nc.tensor.matmul(
                                    pB.rearrange("p (h d) -> p h d", h=H),
                                    lhsT=skT[bs_of(b, j)][:, sb, mb * P:(mb + 1) * P],
                                    rhs=vaT[bs_of(b, j)][:, sb, :, 0:D],
                                    start=(n == 0), stop=(n == B * NB - 1),
                                )
                                n += 1
                        nc.scalar.activation(bt_t[:, mb, :], pB, AF.Copy)

            # ---- z prefix (AG1 has landed by now) ----
            with tc.tile_pool(name="pz", bufs=1) as pz, \
                 tc.tile_pool(name="psz", bufs=1, space="PSUM") as psz:
                Z = pz.tile([NC * BS, HD], f32, tag="z")
                nc.sync.dma_start(out=Z, in_=cs_out)
                zp = psz.tile([BS, HD], f32, tag="zp")
                nc.tensor.matmul(zp, lhsT=ZM, rhs=Z, start=True, stop=True)
                zr16 = pz.tile([BS, HD], bf, tag="zr16")
                nc.scalar.activation(zr16, zp, AF.Copy, bias=1.0 / D)
                nc.sync.dma_start(out=zrow_d, in_=zr16)
                for kb in range(NB):
                    zc = psz.tile([P, BS], f32, tag="zc")
                    nc.tensor.matmul(zc, lhsT=Z[:, kb * P:(kb + 1) * P], rhs=ZM,
                                     start=True, stop=True)
                    nc.scalar.activation(ZCOL[:, kb, :], zc, AF.Copy, bias=1.0 / D)

            # ---- d, skd for BOTH segments first (DVE), then A_t (tensor) ----
            # skd reuses the st tiles (dead until sweep Q writes st)
            with tc.tile_pool(name="pdw", bufs=2) as pdw, \
                 tc.tile_pool(name="psab", bufs=2, space="PSUM") as psab, \
                 tc.tile_pool(name="pstr", bufs=2, space="PSUM") as pstr:
                skd = stT
                for j in range(SPC):
                    for b in range(B):
                        i = bs_of(b, j)
                        sk_i = skT[i]
                        sd = skd[i]
                        dcol = pdw.tile([P, NB], f32, tag="d")
                        rcd = pdw.tile([P, NB], f32, tag="rcd")
                        jnk = pdw.tile([P, HD], bf, tag="jnk", bufs=1)
                        zbp = pdw.tile([P, HD], bf, tag="zbp")
                        nc.gpsimd.dma_start(
                            out=zbp,
                            in_=zrow_d[i:i + 1, :].partition_broadcast(P))
                        for sb in range(NB):
                            nc.vector.tensor_mul(jnk, sk_i[:, sb, :], zbp)
                            nc.vector.tensor_reduce(
                                out=dcol[:, sb:sb + 1], in_=jnk,
                                axis=mybir.AxisListType.X, op=OP.add,
                            )
                        nc.vector.reciprocal(rcd, dcol)
                        for sb in range(NB):
                            nc.vector.tensor_scalar_mul(
                                sd[:, sb, :], sk_i[:, sb, :], rcd[:, sb:sb + 1]
                            )
                for j in range(SPC):
                    at_t = at_ts[j]
                    # A_t = I - sum sk^T skd : upper-triangular blocks only
                    for mb in range(NB):
                        q0 = mb * P
                        pA = psab.tile([P, HD], f32, tag="pp")
                        n = 0
                        for b in range(B):
                            for sb in range(NB):
                                nc.tensor.matmul(
                                    pA[:, 0:HD - q0],
                                    lhsT=skT[bs_of(b, j)][:, sb, q0:q0 + P],
                                    rhs=skd[bs_of(b, j)][:, sb, q0:HD],
                                    start=(n == 0), stop=(n == B * NB - 1),
                                )
                                n += 1
                        nc.scalar.activation(at_t[:, mb, q0:HD], pA[:, 0:HD - q0],
                                             AF.Copy, scale=-1.0)
                        # diagonal block += I
                        nc.vector.tensor_add(
                            at_t[:, mb, q0:q0 + P], at_t[:, mb, q0:q0 + P], ID)
                    # mirror lower blocks by symmetry
                    for mb in range(1, NB):
                        for kb in range(mb):
                            ptr = pstr.tile([P, P], bf, tag="tr")
                            nc.tensor.transpose(
                                ptr, at_t[:, kb, mb * P:(mb + 1) * P], ID)
                            nc.vector.tensor_copy(at_t[:, mb, kb * P:(kb + 1) * P], ptr)

            # ---- pair composition: abA = A0 A1 ; abB = A1 B0 + B1 ----
            abA = pab.tile([P, NB, HD], bf, tag="abA")
            abB = pab.tile([P, NB, HD], bf, tag="abB")
            with tc.tile_pool(name="pscp", bufs=2, space="PSUM") as pscp:
                for mb in range(NB):
                    pA = pscp.tile([P, HD], f32, tag="pp")
                    for kb in range(NB):
                        nc.tensor.matmul(
                            pA, lhsT=AT0[:, kb, mb * P:(mb + 1) * P],
                            rhs=at1[:, kb, :],
                            start=(kb == 0), stop=(kb == NB - 1),
                        )
                    nc.scalar.activation(abA[:, mb, :], pA, AF.Copy)
                for mb in range(NB):
                    pB = pscp.tile([P, HD], f32, tag="pp")
                    for kb in range(NB):
                        nc.tensor.matmul(
                            pB, lhsT=at1[:, kb, mb * P:(mb + 1) * P],
                            rhs=BT0[:, kb, :],
                            start=(kb == 0), stop=(kb == NB - 1),
                        )
                    nc.vector.tensor_add(abB[:, mb, :], pB, bt1[:, mb, :])
            nc.sync.dma_start(out=ab_in[0].rearrange("(kb p) n -> p kb n", p=P), in_=abA)
            nc.sync.dma_start(out=ab_in[1].rearrange("(kb p) n -> p kb n", p=P), in_=abB)

        # ======== AG2 (hidden under sweep Q/attention) ========
        nc.gpsimd.collective_compute(
            "AllGather", OP.bypass,
            replica_groups=[list(range(NC))],
            ins=[ab_in.opt()], outs=[ab_out.opt()],
        )

        # ======== sweep Q: q projection, sq, memread denominator, attention =
        with tc.tile_pool(name="pxq", bufs=2) as pxq, \
             tc.tile_pool(name="pqh", bufs=2) as pqh, \
             tc.tile_pool(name="pw", bufs=9) as pw, \
             tc.tile_pool(name="psq", bufs=2, space="PSUM") as psq, \
             tc.tile_pool(name="psc", bufs=2, space="PSUM") as psc, \
             tc.tile_pool(name="psa", bufs=3, space="PSUM") as psa, \
             tc.tile_pool(name="psd", bufs=1, space="PSUM") as psd:
            for j in range(SPC):
                for b in range(B):
                    i = bs_of(b, j)
                    XT = pxq.tile([P, NB, SEG], bf, tag="xt")
                    nc.sync.dma_start(out=XT, in_=xt_d[b, j].rearrange("kb p s -> p kb s"))
                    qh = pqh.tile([P, NB, SEG], bf, tag="qh")
                    sq_i = sqT[i]
                    for mb in range(NB):
                        pq = psq.tile([P, SEG], f32, tag="pp")
                        for kb in range(NB):
                            nc.tensor.matmul(
                                pq, lhsT=WQ[:, kb, mb * P:(mb + 1) * P],
                                rhs=XT[:, kb, :],
                                start=(kb == 0), stop=(kb == NB - 1),
                            )
                        nc.scalar.activation(qh[:, mb, :], pq, AF.Copy)
                        em = pxq.tile([P, SEG], bf, tag="em")
                        nc.vector.tensor_scalar_min(em, pq, 0.0)
                        ee = pxq.tile([P, SEG], bf, tag="ee")
                        nc.scalar.activation(ee, em, AF.Exp)
                        nc.vector.scalar_tensor_tensor(
                            out=sq_i[:, mb, :], in0=pq, scalar=1.0, in1=ee,
                            op0=OP.add, op1=OP.max,
                        )
                    # memread denominator sq @ z -> DN row 64+i
                    pd = psd.tile([1, SEG], f32, tag="dn")
                    for kb in range(NB):
                        nc.tensor.matmul(
                            pd, lhsT=ZCOL[:, kb, i:i + 1], rhs=sq_i[:, kb, :],
                            start=(kb == 0), stop=(kb == NB - 1),
                        )
                    dtmp = pw.tile([1, SEG], f32, tag="dtmp", bufs=3)
                    nc.scalar.activation(dtmp, pd, AF.Copy)
                    nc.sync.dma_start(
                        out=DN[BS * H + i:BS * H + i + 1, :], in_=dtmp)

                    # ---- attention, one head of score-lead ----
                    kh_i = khT[i]
                    va = vaT[i]
                    st_i = stT[i]
                    wts = [None] * H
                    pats = [None] * H

                    def emit_scores(h):
                        hb, ho = h // 2, (h % 2) * 64
                        wth = []
                        wts[h] = wth
                        for kb in range(NB):
                            q0 = kb * P
                            qf = SEG - q0
                            ps_ = psc.tile([P, SEG], f32, tag="sc")
                            nc.tensor.matmul(
                                ps_[:, 0:qf],
                                lhsT=kh_i[ho:ho + 64, hb, q0:q0 + P],
                                rhs=qh[ho:ho + 64, hb, q0:SEG],
                                start=True, stop=True,
                            )
                            wt = pw.tile([P, SEG], bf, tag="wt")
                            wth.append(wt)
                            nc.scalar.activation(
                                wt[:, 0:qf], ps_[:, 0:qf], AF.Exp, scale=0.125)
                            # causal mask on the diagonal block
                            # (column 0 of this wt is global q = q0)
                            nc.vector.tensor_mul(
                                wt[:, 0:P], wt[:, 0:P], CM)

                    def emit_pat(h):
                        hb, ho = h // 2, (h % 2) * 64
                        wth = wts[h]
                        pat = psa.tile([D + 1, SEG], f32, tag="at")
                        pats[h] = pat
                        for kb in range(NB):
                            q0 = kb * P
                            qf = SEG - q0
                            nc.tensor.matmul(
                                pat[:, q0:SEG],
                                lhsT=va[:, kb, h, :],
                                rhs=wth[kb][:, 0:qf],
                                start=(kb == 0), stop=(kb == NB - 1),
                            )
                        # denominator row out, numerator scaled by (1-g)
                        r = H * i + h
                        dtmp = pw.tile([1, SEG], f32, tag="dtmp", bufs=3)
                        nc.scalar.activation(dtmp, pat[D:D + 1, :], AF.Copy)
                        nc.sync.dma_start(out=DN[r:r + 1, :], in_=dtmp)
                        nc.vector.tensor_scalar_mul(
                            st_i[ho:ho + 64, hb, :], pat[0:D, :],
                            OMG[ho:ho + 64, hb:hb + 1])

                    emit_scores(0)
                    for h in range(1, H):
                        emit_scores(h)
                        emit_pat(h - 1)
                    emit_pat(H - 1)

            # one reciprocal for all denominators
            nc.vector.reciprocal(DNR, DN)
            dn16 = pqh.tile([NDN, SEG], bf, tag="dn16")
            nc.vector.tensor_copy(dn16, DNR)
            nc.sync.dma_start(out=dn_d, in_=dn16)

        # NOTE: wt layout detail — emit_scores writes exp(scores) for
        # q>=q0 into wt[:, q0:SEG]; the causal mask multiplies the
        # diagonal block columns [q0, q0+P) of row-block kb.

        # ======== chain + select (normalizes hidden underneath) ========
        nc.vector.memset(MSEL, 0.0)

        def emit_normalize(i):
            # st_i *= broadcast of attention denominator reciprocals
            dnrb = pnr.tile([P, NB, SEG], bf, tag="dnrb")
            for mb in range(NB):
                for half in range(2):
                    r = H * i + 2 * mb + half
                    nc.gpsimd.dma_start(
                        out=dnrb[half * 64:half * 64 + 64, mb, :],
                        in_=dn_d[r:r + 1, :].partition_broadcast(64))
            nc.gpsimd.tensor_mul(stT[i], stT[i], dnrb)

        with tc.tile_pool(name="pch", bufs=3) as pch, \
             tc.tile_pool(name="pnr", bufs=2) as pnr, \
             tc.tile_pool(name="pmc", bufs=2) as pmc, \
             tc.tile_pool(name="psch", bufs=NB, space="PSUM") as psch:
            mprev = None
            for step in range(NC - 1):
                cB = pch.tile([P, NB, HD], bf, tag="cB")
                nc.sync.dma_start(
                    out=cB, in_=ab_out[step, 1].rearrange("(kb p) n -> p kb n", p=P))
                if step == 0:
                    mcur = cB
                else:
                    cA = pch.tile([P, NB, HD], bf, tag="cA")
                    nc.sync.dma_start(
                        out=cA, in_=ab_out[step, 0].rearrange("(kb p) n -> p kb n", p=P))
                    mcur = pmc.tile([P, NB, HD], bf, tag="mc")
                    for mb in range(NB):
                        pM = psch.tile([P, HD], f32, tag="ch")
                        for kb in range(NB):
                            nc.tensor.matmul(
                                pM, lhsT=cA[:, kb, mb * P:(mb + 1) * P],
                                rhs=mprev[:, kb, :],
                                start=(kb == 0), stop=(kb == NB - 1),
                            )
                        nc.vector.tensor_add(mcur[:, mb, :], pM, cB[:, mb, :])
                for mb in range(NB):
                    nc.vector.scalar_tensor_tensor(
                        out=MSEL[:, mb, :], in0=mcur[:, mb, :],
                        scalar=OH[:, step:step + 1], in1=MSEL[:, mb, :],
                        op0=OP.mult, op1=OP.add,
                    )
                mprev = mcur
                # hide a normalize under each chain step
                emit_normalize(step)
            emit_normalize(NC - 1)

            # M at local segment 1: MLOC1 = A0 MSEL + B0
            for mb in range(NB):
                pM = psch.tile([P, HD], f32, tag="ch")
                for kb in range(NB):
                    nc.tensor.matmul(
                        pM, lhsT=AT0[:, kb, mb * P:(mb + 1) * P],
                        rhs=MSEL[:, kb, :],
                        start=(kb == 0), stop=(kb == NB - 1),
                    )
                nc.vector.tensor_add(MLOC1[:, mb, :], pM, BT0[:, mb, :])

        # ======== phase B: memread, combine, Wd (transposed) ========
        with tc.tile_pool(name="pb", bufs=2) as pb, \
             tc.tile_pool(name="psb", bufs=2, space="PSUM") as psb, \
             tc.tile_pool(name="psw", bufs=2, space="PSUM") as psw:
            for j in range(SPC):
                Mt = MSEL if j == 0 else MLOC1
                for b in range(B):
                    i = bs_of(b, j)
                    sq_i = sqT[i]
                    st_i = stT[i]
                    rcmb = pb.tile([P, SEG], bf, tag="rcmb")
                    nc.gpsimd.dma_start(
                        out=rcmb,
                        in_=dn_d[BS * H + i:BS * H + i + 1, :].partition_broadcast(P))
                    for mb in range(NB):
                        pm = psb.tile([P, SEG], f32, tag="mm")
                        for kb in range(NB):
                            nc.tensor.matmul(
                                pm, lhsT=Mt[:, kb, mb * P:(mb + 1) * P],
                                rhs=sq_i[:, kb, :],
                                start=(kb == 0), stop=(kb == NB - 1),
                            )
                        mtmp = pb.tile([P, SEG], bf, tag="mt")
                        nc.vector.scalar_tensor_tensor(
                            out=mtmp, in0=pm, scalar=GC[:, mb:mb + 1],
                            in1=rcmb, op0=OP.mult, op1=OP.mult,
                        )
                        nc.vector.tensor_add(st_i[:, mb, :], st_i[:, mb, :], mtmp)
                    po = psw.tile([D, SEG], f32, tag="wd")
                    for mb in range(NB):
                        nc.tensor.matmul(
                            po, lhsT=WD[:, mb, :], rhs=st_i[:, mb, :],
                            start=(mb == 0), stop=(mb == NB - 1),
                        )
                    ob = pb.tile([D, SEG], f32, tag="ob")
                    nc.scalar.activation(ob, po, AF.Copy)
                    nc.sync.dma_start(out=out_d[b, j], in_=ob)

    nc.compile()
    return nc


def _prep_inputs(x, Wq, Wk, Wv, Wd, beta):
    """Host-side prep: transpose/cast/shard. Returns in_maps (list of 8 dicts)."""
    g = 1.0 / (1.0 + np.exp(-beta.astype(np.float64)))  # (H,)
    g = g.astype(np.float32)
    gcol = np.repeat(g, D).reshape(NB, P).T.copy()      # (P, NB): g[(kb*128+p)//64]
    omg = (1.0 - np.repeat(g, D)).reshape(NB, P).T.copy()

    def wprep(w):
        return np.ascontiguousarray(
            w.reshape(NB, P, w.shape[1]).astype(bf_np))

    wq_a, wk_a, wv_a = wprep(Wq), wprep(Wk), wprep(Wv)
    wd_a = wprep(Wd)
    cmask = np.triu(np.ones((P, P), np.float32)).astype(bf_np)
    ident = np.eye(P, dtype=np.float32).astype(bf_np)

    # x -> per-core transposed blocks: xt[b, j, kb, p, s] = x[b, (2c+j)*SEG+s, kb*P+p]
    xs = x.reshape(B, NSEG, SEG, DIN)
    in_maps = []
    for c in range(NC):
        xloc = xs[:, 2 * c:2 * c + 2]                        # (B, SPC, SEG, DIN)
        xt = xloc.transpose(0, 1, 3, 2)                      # (B, SPC, DIN, SEG)
        xt = np.ascontiguousarray(
            xt.reshape(B, SPC, NB, P, SEG).astype(bf_np))
        # AG1 global row for (t, b): rank t//2 contributes row (t%2)*B + b
        zmask = np.zeros((64, NC), np.float32)
        for jj in range(NC):
            tgt = 2 * c + (jj // B)
            bb = jj % B
            for t in range(NSEG):
                if t < tgt:
                    zmask[(t // 2) * BS + (t % 2) * B + bb, jj] = 1.0
        oh = np.zeros((P, NC), np.float32)
        if c >= 1:
            oh[:, c - 1] = 1.0
        in_maps.append({
            "xt": xt, "wq": wq_a, "wk": wk_a, "wv": wv_a, "wd": wd_a,
            "gcol": gcol, "omg": omg, "zmask": zmask, "oh": oh,
            "cmask": cmask, "ident": ident,
        })
    return in_maps


def kernel(x, Wq, Wk, Wv, Wd, beta, _trace=False):
    x = np.asarray(x, np.float32)
    in_maps = _prep_inputs(
        x, np.asarray(Wq, np.float32), np.asarray(Wk, np.float32),
        np.asarray(Wv, np.float32), np.asarray(Wd, np.float32),
        np.asarray(beta, np.float32))
    if "nc" not in _CACHE:
        _CACHE["nc"] = _build()
    nc = _CACHE["nc"]
    res = bass_utils.run_bass_kernel_spmd(
        nc, in_maps, core_ids=list(range(NC)), trace=_trace)
    _CACHE["last_results"] = res
    out = np.empty((B, L, D), np.float32)
    for c in range(NC):
        oc = res.results[c]["out"]                  # (B, SPC, D, SEG)
        out[:, 2 * c * SEG:(2 * c + 2) * SEG, :] = (
            oc.transpose(0, 1, 3, 2).reshape(B, SPC * SEG, D))
    return out


# revision 35
# speedup vs baseline: 1.0274x; 1.0274x over previous
"""Trainium2 Bass kernel for nn_MMHA_78039555768536.

Gated mix of per-segment causal softmax attention and a linear-attention
memory (delta rule, memory summed over batch per segment).

Strategy (8 cores): reformulate the memory recurrence as a linear matrix
recurrence  M_{t+1} = A_t M_t + B_t  with
    A_t = I - sum_b sk_b^T diag(1/d_b) sk_b   (symmetric)
    B_t = sum_b sk_b^T v_b
Core c owns segments {2c, 2c+1}.  Two all-gathers:
 AG1: per-segment colsums of sk (for the z prefix)  [tiny, hidden under
      the v-projection sweep]
 AG2: per-core pair composition (Abar^T, Bbar)      [1 MB bf16 per rank,
      hidden under the q-projection + attention sweep]
Every core redundantly runs the 7-step pair chain and selects its own
prefix M via a per-core one-hot (SPMD, no branches).

v2 perf notes vs the first working version:
 - all softmax/memread denominators collected into one [72,512] tile and
   inverted with a single DVE reciprocal (was 80 reciprocals at 3us each)
 - denominator broadcast via batched gpsimd DMAs, overlapped with the
   serial chain
 - k projected once (transposed), k-natural recovered with PE transposes
 - A_t computed upper-triangular only (symmetry), identity folded in so
   the chain needs no ID matmuls; M+B additions fused into DVE reads of
   PSUM
 - Wd projection emitted transposed (free dim 512); host un-transposes
 - attention softly pipelined one head ahead to keep the PE p-state high
"""

import sys

sys.path.insert(0, "/opt/trn_rl_repo")

from contextlib import ExitStack

import numpy as np
import ml_dtypes

import concourse.bass as bass
import concourse.bacc as bacc
import concourse.tile as tile
from concourse import mybir
from concourse import bass_utils

B, L, DIN = 4, 8192, 512
H, D, SEG = 8, 64, 512
HD = H * D
NSEG = L // SEG          # 16
NC = 8                   # cores
SPC = NSEG // NC         # segments per core = 2
P = 128
NB = HD // P             # 4 blocks of 128
BS = B * SPC             # batch-segment units per core = 8

bf = mybir.dt.bfloat16
f32 = mybir.dt.float32
AF = mybir.ActivationFunctionType
OP = mybir.AluOpType
bf_np = ml_dtypes.bfloat16

_CACHE = {}


def _build():
    nc = bacc.Bacc(
        "TRN2",
        target_bir_lowering=False,
        debug=False,
        enable_asserts=False,
        num_devices=NC,
    )

    # ---------------- DRAM I/O ----------------
    xt_d = nc.dram_tensor("xt", [B, SPC, NB, P, SEG], bf, kind="ExternalInput").ap()
    wq_d = nc.dram_tensor("wq", [NB, P, HD], bf, kind="ExternalInput").ap()
    wk_d = nc.dram_tensor("wk", [NB, P, HD], bf, kind="ExternalInput").ap()
    wv_d = nc.dram_tensor("wv", [NB, P, HD], bf, kind="ExternalInput").ap()
    wd_d = nc.dram_tensor("wd", [NB, P, D], bf, kind="ExternalInput").ap()
    gcol_d = nc.dram_tensor("gcol", [P, NB], f32, kind="ExternalInput").ap()
    omg_d = nc.dram_tensor("omg", [P, NB], f32, kind="ExternalInput").ap()
    zmask_d = nc.dram_tensor("zmask", [64, NC], f32, kind="ExternalInput").ap()
    oh_d = nc.dram_tensor("oh", [P, NC], f32, kind="ExternalInput").ap()
    mask_d = nc.dram_tensor("cmask", [P, P], bf, kind="ExternalInput").ap()
    ident_d = nc.dram_tensor("ident", [P, P], bf, kind="ExternalInput").ap()
    # transposed output: [D, SEG] per (b, j); host un-transposes
    out_d = nc.dram_tensor("out", [B, SPC, D, SEG], f32, kind="ExternalOutput").ap()

    NDN = BS * H + BS    # 72 denominator rows (64 attn + 8 memread)

    with tile.TileContext(nc) as tc, ExitStack() as ctx:
        const = ctx.enter_context(tc.tile_pool(name="const", bufs=1))
        dram = ctx.enter_context(tc.tile_pool(name="dram", bufs=1, space="DRAM"))
        sing = ctx.enter_context(tc.tile_pool(name="sing", bufs=1))

        WQ = const.tile([P, NB, HD], bf)
        WK = const.tile([P, NB, HD], bf)
        WV = const.tile([P, NB, HD], bf)
        WD = const.tile([P, NB, D], bf)
        GC = const.tile([P, NB], f32)
        OMG = const.tile([P, NB], f32)
        ZM = const.tile([64, NC], f32)
        OH = const.tile([P, NC], f32)
        CM = const.tile([P, P], bf)
        ID = const.tile([P, P], bf)
        ONE = const.tile([P, 1], bf)

        nc.sync.dma_start(out=WQ, in_=wq_d.rearrange("kb p n -> p kb n"))
        nc.sync.dma_start(out=WK, in_=wk_d.rearrange("kb p n -> p kb n"))
        nc.sync.dma_start(out=WV, in_=wv_d.rearrange("kb p n -> p kb n"))
        nc.sync.dma_start(out=WD, in_=wd_d.rearrange("kb p n -> p kb n"))
        nc.sync.dma_start(out=GC, in_=gcol_d)
        nc.sync.dma_start(out=OMG, in_=omg_d)
        nc.sync.dma_start(out=ZM, in_=zmask_d)
        nc.sync.dma_start(out=OH, in_=oh_d)
        nc.sync.dma_start(out=CM, in_=mask_d)
        nc.sync.dma_start(out=ID, in_=ident_d)
        nc.vector.memset(ONE, 1.0)

        # collective bounce buffers
        cs_in = dram.tile([BS, HD], f32)
        cs_out = dram.tile([NC * BS, HD], f32, addr_space="Shared")
        ab_in = dram.tile([2, HD, HD], bf)
        ab_out = dram.tile([NC, 2, HD, HD], bf, addr_space="Shared")
        zrow_d = dram.tile([BS, HD], bf)
        dn_d = dram.tile([NDN, SEG], bf)

        # cross-phase singles
        ZCOL = sing.tile([P, NB, BS], bf)
        AT0 = sing.tile([P, NB, HD], bf)    # I - K of local segment 0
        BT0 = sing.tile([P, NB, HD], bf)
        MSEL = sing.tile([P, NB, HD], bf)   # selected M at segment 2c
        MLOC1 = sing.tile([P, NB, HD], bf)  # M at segment 2c+1
        DN = sing.tile([NDN, SEG], f32)     # raw denominators
        DNR = sing.tile([NDN, SEG], f32)    # reciprocals

        def bs_of(b, j):
            return j * B + b

        # persistent per-bs tensors (sk slots are reused as sq in sweep 2)
        keep = ctx.enter_context(tc.tile_pool(name="keep", bufs=BS))
        skT = [keep.tile([P, NB, SEG], bf, tag="sk", name=f"sk{i}") for i in range(BS)]
        khT = [keep.tile([P, NB, SEG], bf, tag="kh", name=f"kh{i}") for i in range(BS)]
        vaT = [keep.tile([P, NB, H, D + 1], bf, tag="va", name=f"va{i}")
               for i in range(BS)]
        stT = [keep.tile([P, NB, SEG], bf, tag="st", name=f"st{i}") for i in range(BS)]
        sqT = skT  # reuse storage: sk dead after A/B, sq born in sweep 2

        # ======== sweep K: kT projection, k-nat via PE transpose, sk, cs ====
        with tc.tile_pool(name="pxk", bufs=2) as pxk, \
             tc.tile_pool(name="psk", bufs=2, space="PSUM") as psk, \
             tc.tile_pool(name="pskn", bufs=2, space="PSUM") as pskn, \
             tc.tile_pool(name="pscs", bufs=2, space="PSUM") as pscs:
            for j in range(SPC):
                for b in range(B):
                    i = bs_of(b, j)
                    XT = pxk.tile([P, NB, SEG], bf, tag="xt")
                    nc.sync.dma_start(out=XT, in_=xt_d[b, j].rearrange("kb p s -> p kb s"))
                    kh_i = khT[i]
                    for mb in range(NB):
                        pk = psk.tile([P, SEG], f32, tag="pk")
                        for kb in range(NB):
                            nc.tensor.matmul(
                                pk, lhsT=WK[:, kb, mb * P:(mb + 1) * P],
                                rhs=XT[:, kb, :],
                                start=(kb == 0), stop=(kb == NB - 1),
                            )
                        nc.scalar.activation(kh_i[:, mb, :], pk, AF.Copy)
                    sk_i = skT[i]
                    for sb in range(NB):
                        # k natural block row sb from transposes of kh
                        pkn = pskn.tile([P, SEG], bf, tag="pkn")
                        for mb in range(NB):
                            nc.tensor.transpose(
                                pkn[:, mb * P:(mb + 1) * P],
                                kh_i[:, mb, sb * P:(sb + 1) * P], ID,
                            )
                        # elu1(k) = max(k + 1, exp(min(k, 0)))
                        em = pxk.tile([P, SEG], bf, tag="em")
                        nc.vector.tensor_scalar_min(em, pkn, 0.0)
                        ee = pxk.tile([P, SEG], bf, tag="ee")
                        nc.scalar.activation(ee, em, AF.Exp)
                        nc.vector.scalar_tensor_tensor(
                            out=sk_i[:, sb, :], in0=pkn, scalar=1.0, in1=ee,
                            op0=OP.add, op1=OP.max,
                        )
                    pc = pscs.tile([1, HD], f32, tag="pc")
                    for sb in range(NB):
                        nc.tensor.matmul(
                            pc, lhsT=ONE, rhs=sk_i[:, sb, :],
                            start=(sb == 0), stop=(sb == NB - 1),
                        )
                    cs_sb = pxk.tile([1, HD], f32, tag="cs")
                    nc.scalar.activation(cs_sb, pc, AF.Copy)
                    nc.sync.dma_start(out=cs_in[i:i + 1, :], in_=cs_sb)

        # ======== AG1 (hidden under sweep V) ========
        nc.gpsimd.collective_compute(
            "AllGather", OP.bypass,
            replica_groups=[list(range(NC))],
            ins=[cs_in.opt()], outs=[cs_out.opt()],
        )

        # ======== sweep V: v projection ========
        with tc.tile_pool(name="pxv", bufs=2) as pxv, \
             tc.tile_pool(name="psv", bufs=2, space="PSUM") as psv:
            for j in range(SPC):
                for b in range(B):
                    i = bs_of(b, j)
                    XT = pxv.tile([P, NB, SEG], bf, tag="xt")
                    nc.sync.dma_start(out=XT, in_=xt_d[b, j].rearrange("kb p s -> p kb s"))
                    va = vaT[i]
                    nc.vector.memset(va[:, :, :, D:D + 1], 1.0)
                    for sb in range(NB):
                        pv = psv.tile([P, SEG], f32, tag="pv")
                        for kb in range(NB):
                            nc.tensor.matmul(
                                pv, lhsT=XT[:, kb, sb * P:(sb + 1) * P],
                                rhs=WV[:, kb, :],
                                start=(kb == 0), stop=(kb == NB - 1),
                            )
                        nc.vector.tensor_copy(
                            va[:, sb, :, 0:D], pv.rearrange("p (h d) -> p h d", h=H)
                        )

        # ======== per segment: B_t first (AG1-independent), then z prefix,
        # then d/skd + A_t (triangular) ========
        at1 = bt1 = None
        with tc.tile_pool(name="pab", bufs=1) as pab:
            bt_ts = []
            at_ts = []
            for j in range(SPC):
                at_ts.append(pab.tile([P, NB, HD], bf, tag="at", name=f"at{j}")
                             if j > 0 else AT0)
                bt_ts.append(pab.tile([P, NB, HD], bf, tag="bt", name=f"bt{j}")
                             if j > 0 else BT0)
            at1, bt1 = at_ts[1], bt_ts[1]

            # ---- B_t = sum sk^T v for both segments (fills the AG1 window) --
            with tc.tile_pool(name="psbb", bufs=2, space="PSUM") as psbb:
                for j in range(SPC):
                    bt_t = bt_ts[j]
                    for mb in range(NB):
                        pB = psbb.tile([P, HD], f32, tag="pp")
                        n = 0
                        for b in range(B):
                            for sb in range(NB):
                                nc.tensor.matmul(
                                    pB.rearrange("p (h d) -> p h d", h=H),
                                    lhsT=skT[bs_of(b, j)][:, sb, mb * P:(mb + 1) * P],
                                    rhs=vaT[bs_of(b, j)][:, sb, :, 0:D],
                                    start=(n == 0), stop=(n == B * NB - 1),
                                )
                                n += 1
                        nc.scalar.activation(bt_t[:, mb, :], pB, AF.Copy)

            # ---- z prefix (AG1 has landed by now) ----
            with tc.tile_pool(name="pz", bufs=1) as pz, \
                 tc.tile_pool(name="psz", bufs=1, space="PSUM") as psz:
                Z = pz.tile([NC * BS, HD], f32, tag="z")
                nc.sync.dma_start(out=Z, in_=cs_out)
                zp = psz.tile([BS, HD], f32, tag="zp")
                nc.tensor.matmul(zp, lhsT=ZM, rhs=Z, start=True, stop=True)
                zr16 = pz.tile([BS, HD], bf, tag="zr16")
                nc.scalar.activation(zr16, zp, AF.Copy, bias=1.0 / D)
                nc.sync.dma_start(out=zrow_d, in_=zr16)
                for kb in range(NB):
                    zc = psz.tile([P, BS], f32, tag="zc")
                    nc.tensor.matmul(zc, lhsT=Z[:, kb * P:(kb + 1) * P], rhs=ZM,
                                     start=True, stop=True)
                    nc.scalar.activation(ZCOL[:, kb, :], zc, AF.Copy, bias=1.0 / D)

            # ---- d, skd for BOTH segments first (DVE), then A_t (tensor) ----
            # skd reuses the st tiles (dead until sweep Q writes st)
            with tc.tile_pool(name="pdw", bufs=2) as pdw, \
                 tc.tile_pool(name="psab", bufs=2, space="PSUM") as psab, \
                 tc.tile_pool(name="pstr", bufs=2, space="PSUM") as pstr:
                skd = stT
                for j in range(SPC):
                    for b in range(B):
                        i = bs_of(b, j)
                        sk_i = skT[i]
                        sd = skd[i]
                        dcol = pdw.tile([P, NB], f32, tag="d")
                        rcd = pdw.tile([P, NB], f32, tag="rcd")
                        jnk = pdw.tile([P, HD], bf, tag="jnk", bufs=1)
                        zbp = pdw.tile([P, HD], bf, tag="zbp")
                        nc.gpsimd.dma_start(
                            out=zbp,
                            in_=zrow_d[i:i + 1, :].partition_broadcast(P))
                        for sb in range(NB):
                            nc.vector.tensor_mul(jnk, sk_i[:, sb, :], zbp)
                            nc.vector.tensor_reduce(
                                out=dcol[:, sb:sb + 1], in_=jnk,
                                axis=mybir.AxisListType.X, op=OP.add,
                            )
                        nc.vector.reciprocal(rcd, dcol)
                        for sb in range(NB):
                            nc.vector.tensor_scalar_mul(
                                sd[:, sb, :], sk_i[:, sb, :], rcd[:, sb:sb + 1]
                            )
                for j in range(SPC):
                    at_t = at_ts[j]
                    # A_t = I - sum sk^T skd : upper-triangular blocks only
                    for mb in range(NB):
                        q0 = mb * P
                        pA = psab.tile([P, HD], f32, tag="pp")
                        n = 0
                        for b in range(B):
                            for sb in range(NB):
                                nc.tensor.matmul(
                                    pA[:, 0:HD - q0],
                                    lhsT=skT[bs_of(b, j)][:, sb, q0:q0 + P],
                                    rhs=skd[bs_of(b, j)][:, sb, q0:HD],
                                    start=(n == 0), stop=(n == B * NB - 1),
                                )
                                n += 1
                        nc.scalar.activation(at_t[:, mb, q0:HD], pA[:, 0:HD - q0],
                                             AF.Copy, scale=-1.0)
                        # diagonal block += I
                        nc.vector.tensor_add(
                            at_t[:, mb, q0:q0 + P], at_t[:, mb, q0:q0 + P], ID)
                    # mirror lower blocks by symmetry
                    for mb in range(1, NB):
                        for kb in range(mb):
                            ptr = pstr.tile([P, P], bf, tag="tr")
                            nc.tensor.transpose(
                                ptr, at_t[:, kb, mb * P:(mb + 1) * P], ID)
                            nc.vector.tensor_copy(at_t[:, mb, kb * P:(kb + 1) * P], ptr)

            # ---- pair composition: abA = A0 A1 ; abB = A1 B0 + B1 ----
            abA = pab.tile([P, NB, HD], bf, tag="abA")
            abB = pab.tile([P, NB, HD], bf, tag="abB")
            with tc.tile_pool(name="pscp", bufs=2, space="PSUM") as pscp:
                for mb in range(NB):
                    pA = pscp.tile([P, HD], f32, tag="pp")
                    for kb in range(NB):
                        nc.tensor.matmul(
                            pA, lhsT=AT0[:, kb, mb * P:(mb + 1) * P],
                            rhs=at1[:, kb, :],
                            start=(kb == 0), stop=(kb == NB - 1),
                        )
                    nc.scalar.activation(abA[:, mb, :], pA, AF.Copy)
                for mb in range(NB):
                    pB = pscp.tile([P, HD], f32, tag="pp")
                    for kb in range(NB):
                        nc.tensor.matmul(
                            pB, lhsT=at1[:, kb, mb * P:(mb + 1) * P],
                            rhs=BT0[:, kb, :],
                            start=(kb == 0), stop=(kb == NB - 1),
                        )
                    nc.vector.tensor_add(abB[:, mb, :], pB, bt1[:, mb, :])
            nc.sync.dma_start(out=ab_in[0].rearrange("(kb p) n -> p kb n", p=P), in_=abA)
            nc.sync.dma_start(out=ab_in[1].rearrange("(kb p) n -> p kb n", p=P), in_=abB)

        # prefetch the first sweep-Q x tile while AG2 spins up
        pxq0 = ctx.enter_context(tc.tile_pool(name="pxq0", bufs=1))
        XT00 = pxq0.tile([P, NB, SEG], bf, tag="xt0")
        nc.sync.dma_start(out=XT00, in_=xt_d[0, 0].rearrange("kb p s -> p kb s"))

        # ======== AG2 (hidden under sweep Q/attention) ========
        nc.gpsimd.collective_compute(
            "AllGather", OP.bypass,
            replica_groups=[list(range(NC))],
            ins=[ab_in.opt()], outs=[ab_out.opt()],
        )

        # ======== sweep Q: q projection, sq, memread denominator, attention =
        with tc.tile_pool(name="pxq", bufs=2) as pxq, \
             tc.tile_pool(name="pqh", bufs=2) as pqh, \
             tc.tile_pool(name="pw", bufs=9) as pw, \
             tc.tile_pool(name="psq", bufs=2, space="PSUM") as psq, \
             tc.tile_pool(name="psc", bufs=2, space="PSUM") as psc, \
             tc.tile_pool(name="psa", bufs=3, space="PSUM") as psa, \
             tc.tile_pool(name="psd", bufs=1, space="PSUM") as psd:
            for j in range(SPC):
                for b in range(B):
                    i = bs_of(b, j)
                    if i == 0:
                        XT = XT00
                    else:
                        XT = pxq.tile([P, NB, SEG], bf, tag="xt")
                        nc.sync.dma_start(
                            out=XT, in_=xt_d[b, j].rearrange("kb p s -> p kb s"))
                    qh = pqh.tile([P, NB, SEG], bf, tag="qh")
                    sq_i = sqT[i]
                    for mb in range(NB):
                        pq = psq.tile([P, SEG], f32, tag="pp")
                        for kb in range(NB):
                            nc.tensor.matmul(
                                pq, lhsT=WQ[:, kb, mb * P:(mb + 1) * P],
                                rhs=XT[:, kb, :],
                                start=(kb == 0), stop=(kb == NB - 1),
                            )
                        nc.scalar.activation(qh[:, mb, :], pq, AF.Copy)
                        em = pxq.tile([P, SEG], bf, tag="em")
                        nc.vector.tensor_scalar_min(em, pq, 0.0)
                        ee = pxq.tile([P, SEG], bf, tag="ee")
                        nc.scalar.activation(ee, em, AF.Exp)
                        nc.vector.scalar_tensor_tensor(
                            out=sq_i[:, mb, :], in0=pq, scalar=1.0, in1=ee,
                            op0=OP.add, op1=OP.max,
                        )
                    # memread denominator sq @ z -> DN row 64+i
                    pd = psd.tile([1, SEG], f32, tag="dn")
                    for kb in range(NB):
                        nc.tensor.matmul(
                            pd, lhsT=ZCOL[:, kb, i:i + 1], rhs=sq_i[:, kb, :],
                            start=(kb == 0), stop=(kb == NB - 1),
                        )
                    dtmp = pw.tile([1, SEG], f32, tag="dtmp", bufs=3)
                    nc.scalar.activation(dtmp, pd, AF.Copy)
                    nc.gpsimd.dma_start(
                        out=DN[BS * H + i:BS * H + i + 1, :], in_=dtmp)

                    # ---- attention, one head of score-lead ----
                    kh_i = khT[i]
                    va = vaT[i]
                    st_i = stT[i]
                    wts = [None] * H
                    pats = [None] * H

                    def emit_scores(h):
                        hb, ho = h // 2, (h % 2) * 64
                        wth = []
                        wts[h] = wth
                        for kb in range(NB):
                            q0 = kb * P
                            qf = SEG - q0
                            ps_ = psc.tile([P, SEG], f32, tag="sc")
                            nc.tensor.matmul(
                                ps_[:, 0:qf],
                                lhsT=kh_i[ho:ho + 64, hb, q0:q0 + P],
                                rhs=qh[ho:ho + 64, hb, q0:SEG],
                                start=True, stop=True,
                            )
                            wt = pw.tile([P, SEG], bf, tag="wt")
                            wth.append(wt)
                            nc.scalar.activation(
                                wt[:, 0:qf], ps_[:, 0:qf], AF.Exp, scale=0.125)
                            # causal mask on the diagonal block
                            # (column 0 of this wt is global q = q0)
                            nc.vector.tensor_mul(
                                wt[:, 0:P], wt[:, 0:P], CM)

                    def emit_pat(h):
                        hb, ho = h // 2, (h % 2) * 64
                        wth = wts[h]
                        pat = psa.tile([D + 1, SEG], f32, tag="at")
                        pats[h] = pat
                        for kb in range(NB):
                            q0 = kb * P
                            qf = SEG - q0
                            nc.tensor.matmul(
                                pat[:, q0:SEG],
                                lhsT=va[:, kb, h, :],
                                rhs=wth[kb][:, 0:qf],
                                start=(kb == 0), stop=(kb == NB - 1),
                            )
                        # denominator row out, numerator scaled by (1-g)
                        r = H * i + h
                        dtmp = pw.tile([1, SEG], f32, tag="dtmp", bufs=3)
                        if h % 2 == 0:
                            nc.scalar.activation(dtmp, pat[D:D + 1, :], AF.Copy)
                        else:
                            nc.vector.tensor_copy(dtmp, pat[D:D + 1, :])
                        nc.gpsimd.dma_start(out=DN[r:r + 1, :], in_=dtmp)
                        nc.vector.tensor_scalar_mul(
                            st_i[ho:ho + 64, hb, :], pat[0:D, :],
                            OMG[ho:ho + 64, hb:hb + 1])

                    emit_scores(0)
                    for h in range(1, H):
                        emit_scores(h)
                        emit_pat(h - 1)
                    emit_pat(H - 1)

            # one reciprocal for all denominators
            nc.vector.reciprocal(DNR, DN)
            dn16 = pqh.tile([NDN, SEG], bf, tag="dn16")
            nc.vector.tensor_copy(dn16, DNR)
            nc.gpsimd.dma_start(out=dn_d, in_=dn16)

        # NOTE: wt layout detail — emit_scores writes exp(scores) for
        # q>=q0 into wt[:, q0:SEG]; the causal mask multiplies the
        # diagonal block columns [q0, q0+P) of row-block kb.

        # ======== chain + select (normalizes hidden underneath) ========
        nc.vector.memset(MSEL, 0.0)

        def emit_nbcast(i):
            # broadcast attention denominator reciprocals for st_i
            dnrb = pnr.tile([P, NB, SEG], bf, tag="dnrb")
            for mb in range(NB):
                for half in range(2):
                    r = H * i + 2 * mb + half
                    nc.gpsimd.dma_start(
                        out=dnrb[half * 64:half * 64 + 64, mb, :],
                        in_=dn_d[r:r + 1, :].partition_broadcast(64))
            return dnrb

        with tc.tile_pool(name="pch", bufs=3) as pch, \
             tc.tile_pool(name="pnr", bufs=2) as pnr, \
             tc.tile_pool(name="pmc", bufs=2) as pmc, \
             tc.tile_pool(name="psch", bufs=NB, space="PSUM") as psch:
            mprev = None
            dnrbs = {}
            for step in range(NC - 1):
                cB = pch.tile([P, NB, HD], bf, tag="cB")
                nc.sync.dma_start(
                    out=cB, in_=ab_out[step, 1].rearrange("(kb p) n -> p kb n", p=P))
                if step == 0:
                    mcur = cB
                else:
                    cA = pch.tile([P, NB, HD], bf, tag="cA")
                    nc.sync.dma_start(
                        out=cA, in_=ab_out[step, 0].rearrange("(kb p) n -> p kb n", p=P))
                    mcur = pmc.tile([P, NB, HD], bf, tag="mc")
                    for mb in range(NB):
                        pM = psch.tile([P, HD], f32, tag="ch")
                        for kb in range(NB):
                            nc.tensor.matmul(
                                pM, lhsT=cA[:, kb, mb * P:(mb + 1) * P],
                                rhs=mprev[:, kb, :],
                                start=(kb == 0), stop=(kb == NB - 1),
                            )
                        nc.vector.tensor_add(mcur[:, mb, :], pM, cB[:, mb, :])
                for mb in range(NB):
                    nc.vector.scalar_tensor_tensor(
                        out=MSEL[:, mb, :], in0=mcur[:, mb, :],
                        scalar=OH[:, step:step + 1], in1=MSEL[:, mb, :],
                        op0=OP.mult, op1=OP.add,
                    )
                mprev = mcur
                # prefetch one normalize broadcast per step; the DVE multiply
                # trails one step behind so it never leads the chain adds
                dnrbs[step] = emit_nbcast(step)
                if step >= 1:
                    nc.vector.tensor_mul(
                        stT[step - 1], stT[step - 1], dnrbs.pop(step - 1))
            dnrbs[NC - 1] = emit_nbcast(NC - 1)
            for i2 in (NC - 2, NC - 1):
                nc.vector.tensor_mul(stT[i2], stT[i2], dnrbs.pop(i2))

            # M at local segment 1: MLOC1 = A0 MSEL + B0
            for mb in range(NB):
                pM = psch.tile([P, HD], f32, tag="ch")
                for kb in range(NB):
                    nc.tensor.matmul(
                        pM, lhsT=AT0[:, kb, mb * P:(mb + 1) * P],
                        rhs=MSEL[:, kb, :],
                        start=(kb == 0), stop=(kb == NB - 1),
                    )
                nc.vector.tensor_add(MLOC1[:, mb, :], pM, BT0[:, mb, :])

        # ======== phase B: memread, combine, Wd (transposed) ========
        with tc.tile_pool(name="pb", bufs=2) as pb, \
             tc.tile_pool(name="psb", bufs=2, space="PSUM") as psb, \
             tc.tile_pool(name="psw", bufs=2, space="PSUM") as psw:
            for j in range(SPC):
                Mt = MSEL if j == 0 else MLOC1
                for b in range(B):
                    i = bs_of(b, j)
                    sq_i = sqT[i]
                    st_i = stT[i]
                    rcmb = pb.tile([P, SEG], bf, tag="rcmb")
                    nc.gpsimd.dma_start(
                        out=rcmb,
                        in_=dn_d[BS * H + i:BS * H + i + 1, :].partition_broadcast(P))
                    for mb in range(NB):
                        pm = psb.tile([P, SEG], f32, tag="mm")
                        for kb in range(NB):
                            nc.tensor.matmul(
                                pm, lhsT=Mt[:, kb, mb * P:(mb + 1) * P],
                                rhs=sq_i[:, kb, :],
                                start=(kb == 0), stop=(kb == NB - 1),
                            )
                        mtmp = pb.tile([P, SEG], bf, tag="mt")
                        nc.vector.scalar_tensor_tensor(
                            out=mtmp, in0=pm, scalar=GC[:, mb:mb + 1],
                            in1=rcmb, op0=OP.mult, op1=OP.mult,
                        )
                        nc.vector.tensor_add(st_i[:, mb, :], st_i[:, mb, :], mtmp)
                    po = psw.tile([D, SEG], f32, tag="wd")
                    for mb in range(NB):
                        nc.tensor.matmul(
                            po, lhsT=WD[:, mb, :], rhs=st_i[:, mb, :],
                            start=(mb == 0), stop=(mb == NB - 1),
                        )
                    ob = pb.tile([D, SEG], f32, tag="ob")
                    nc.scalar.activation(ob, po, AF.Copy)
                    nc.sync.dma_start(out=out_d[b, j], in_=ob)

    nc.compile()
    return nc


def _prep_inputs(x, Wq, Wk, Wv, Wd, beta):
    """Host-side prep: transpose/cast/shard. Returns in_maps (list of 8 dicts)."""
    g = 1.0 / (1.0 + np.exp(-beta.astype(np.float64)))  # (H,)
    g = g.astype(np.float32)
    gcol = np.repeat(g, D).reshape(NB, P).T.copy()      # (P, NB): g[(kb*128+p)//64]
    omg = (1.0 - np.repeat(g, D)).reshape(NB, P).T.copy()

    def wprep(w):
        return np.ascontiguousarray(
            w.reshape(NB, P, w.shape[1]).astype(bf_np))

    wq_a, wk_a, wv_a = wprep(Wq), wprep(Wk), wprep(Wv)
    wd_a = wprep(Wd)
    cmask = np.triu(np.ones((P, P), np.float32)).astype(bf_np)
    ident = np.eye(P, dtype=np.float32).astype(bf_np)

    # x -> per-core transposed blocks: xt[b, j, kb, p, s] = x[b, (2c+j)*SEG+s, kb*P+p]
    xs = x.reshape(B, NSEG, SEG, DIN)
    in_maps = []
    for c in range(NC):
        xloc = xs[:, 2 * c:2 * c + 2]                        # (B, SPC, SEG, DIN)
        xt = xloc.transpose(0, 1, 3, 2)                      # (B, SPC, DIN, SEG)
        xt = np.ascontiguousarray(
            xt.reshape(B, SPC, NB, P, SEG).astype(bf_np))
        # AG1 global row for (t, b): rank t//2 contributes row (t%2)*B + b
        zmask = np.zeros((64, NC), np.float32)
        for jj in range(NC):
            tgt = 2 * c + (jj // B)
            bb = jj % B
            for t in range(NSEG):
                if t < tgt:
                    zmask[(t // 2) * BS + (t % 2) * B + bb, jj] = 1.0
        oh = np.zeros((P, NC), np.float32)
        if c >= 1:
            oh[:, c - 1] = 1.0
        in_maps.append({
            "xt": xt, "wq": wq_a, "wk": wk_a, "wv": wv_a, "wd": wd_a,
            "gcol": gcol, "omg": omg, "zmask": zmask, "oh": oh,
            "cmask": cmask, "ident": ident,
        })
    return in_maps


def kernel(x, Wq, Wk, Wv, Wd, beta, _trace=False):
    x = np.asarray(x, np.float32)
    in_maps = _prep_inputs(
        x, np.asarray(Wq, np.float32), np.asarray(Wk, np.float32),
        np.asarray(Wv, np.float32), np.asarray(Wd, np.float32),
        np.asarray(beta, np.float32))
    if "nc" not in _CACHE:
        _CACHE["nc"] = _build()
    nc = _CACHE["nc"]
    res = bass_utils.run_bass_kernel_spmd(
        nc, in_maps, core_ids=list(range(NC)), trace=_trace)
    _CACHE["last_results"] = res
    out = np.empty((B, L, D), np.float32)
    for c in range(NC):
        oc = res.results[c]["out"]                  # (B, SPC, D, SEG)
        out[:, 2 * c * SEG:(2 * c + 2) * SEG, :] = (
            oc.transpose(0, 1, 3, 2).reshape(B, SPC * SEG, D))
    return out
